# revision 5
# baseline (speedup 1.0000x reference)
"""CANLayer (GNN message passing) Trainium2 kernel — 8 NeuronCores.

y = sigmoid(L_down @ (x Wc) + L_up @ (x Wc) + x Wl)

v2 design (self-contained: full inputs in, full output out):
  - segment_sum commutes with the dense right-multiplication by Wc: gather raw
    x rows per COO entry, scale by val, segment-sum per 128-row dest block on
    the PE, then apply Wc/Wl per block.
  - dest rows sharded across 8 cores (12500 each); entries bucketed by
    (superblock of 4 dest blocks, source quarter, block) on the host.
  - gather: 128B elements (64ch fp16) from a 256B-pitch table — half the HBM
    traffic of the 256B-elem minimum that bass' dma_gather wrapper enforces
    (the Q7 kernel itself only needs the 256B stride alignment).
  - gathers for quarter q issue on SWDGE queue q: queues 1-3 run their
    descriptor generation on GPSIMD core-pairs (2,3),(4,5),(6,7) in the
    background, so 4 quarters' descgen runs concurrently (~8ns/desc/pair).
  - one-hot S[e, pos] = (iota == rloc_e) is built in BATCHED tensor_tensor
    is_equal ops over a quarter-superblock span using stride-0 broadcast APs
    for rloc — ~2 DVE cycles/elem instead of a per-tile tensor_scalar.
  - val is folded into the gathered rows (G2 = G * val, batched broadcast
    mult), so the one-hot is a pure 0/1 matrix.
  - PE per entry-tile: psum_sT[64, 4*128] += G2_t[:, :64].T @ S_t (one
    accumulation group per superblock bank).
"""
import os

import numpy as np

import concourse.mybir as mybir
import concourse.tile as tile
from concourse import bacc
from concourse import bass_utils
from concourse import ap_utils
from concourse._compat import round_up_to_multiple

N = 100000
C = 64
NCORES = 8
P = 128
R = N // NCORES            # 12500 rows per core
NBLK = (R + P - 1) // P    # 98 blocks per core
RPAD = NBLK * P            # 12544
NQ = 4
QROWS = N // NQ            # 25000
SB = 4                     # dest blocks per superblock (one PSUM bank)
NSB = (NBLK + SB - 1) // SB


def _exact_div(a, b):
    assert a % b == 0
    return a // b


def _dma_gather_128(eng, out_ap, in_ap, idxs_ap, num_idxs, elem_size,
                    elem_step, queue_num=0):
    """dma_gather with sub-256B elements (stride still 256B-aligned).

    Mirrors concourse.bass dma_gather minus its `elem_size_bytes % 256`
    assert, which is a transpose-path restriction the non-transpose Q7
    kernel does not need (verified bit-exact vs the 256B path on HW).
    """
    assert idxs_ap.dtype == mybir.dt.int16
    assert in_ap.dtype == out_ap.dtype
    assert ap_utils.ap_is_contiguous(out_ap.ap[1:])
    assert ap_utils.ap_is_contiguous(idxs_ap.ap[1:])
    assert in_ap.ap[-1][1] == elem_size == out_ap.ap[-1][1]
    assert out_ap.ap[0][1] * out_ap.ap[1][1] == round_up_to_multiple(num_idxs, 128)
    assert in_ap.ap[0][0] == elem_step
    stride_bytes = elem_step * mybir.dt.size(in_ap.dtype)
    stride_bytes_256 = _exact_div(stride_bytes, 256)
    _in_ap = eng.lower_ap_dma(in_ap, for_custom_bir_dma=True)
    return eng.add_instruction(
        mybir.InstDMAGatherAnt(
            name=eng.bass.get_next_instruction_name(),
            ins=[*_in_ap, eng.lower_ap(idxs_ap),
                 eng.lower_val_access(eng.to_reg(num_idxs))],
            outs=[eng.lower_ap(out_ap)],
            transpose=False, num_idxs=num_idxs, elem_size=elem_size,
            stride_bytes_256=stride_bytes_256, gen_mode=0,
            single_packet=False, queue_num=queue_num,
            sbuf_tokens_per_rank=0, sbuf_free_dim_per_rank=0,
            sbuf_free_dim_pad_per_rank=0, sbuf_byte_offset=0))


# ---------------------------------------------------------------- host prep

def _preprocess(inputs):
    x = np.ascontiguousarray(np.asarray(inputs["x"], dtype=np.float32))
    w_conv = np.asarray(inputs["w_conv"], dtype=np.float32)
    w_lin = np.asarray(inputs["w_lin"], dtype=np.float32)

    rows = np.concatenate([np.asarray(inputs["down_rows"]),
                           np.asarray(inputs["up_rows"])]).astype(np.int64)
    cols = np.concatenate([np.asarray(inputs["down_cols"]),
                           np.asarray(inputs["up_cols"])]).astype(np.int64)
    vals = np.concatenate([np.asarray(inputs["down_vals"]),
                           np.asarray(inputs["up_vals"])]).astype(np.float32)

    core = rows // R
    rl = rows % R
    blk = rl // P
    rloc = rl - blk * P
    q = cols // QROWS
    qcol = cols - q * QROWS

    # group order: (core, superblock, quarter, block-in-superblock)
    sb = blk // SB
    bin_ = blk - sb * SB
    gkey = (sb * NQ + q) * SB + bin_            # within-core group id
    ngpc = NSB * NQ * SB                        # groups per core (incl ghosts)
    key = core * ngpc + gkey
    order = np.argsort(key, kind="stable")
    key_s = key[order]
    qcol_s = qcol[order]
    rloc_s = rloc[order]
    vals_s = vals[order]

    ngroups = NCORES * ngpc
    counts = np.bincount(key_s, minlength=ngroups).reshape(NCORES, ngpc)
    caps = counts.max(axis=0)                   # [ngpc]
    # ghost groups (blocks beyond NBLK in the last superblock) stay size 0
    g_ids = np.arange(ngpc)
    g_blk = (g_ids // (NQ * SB)) * SB + (g_ids % SB)
    ghost = g_blk >= NBLK
    caps = np.where(ghost, 0, np.maximum(((caps + P - 1) // P) * P, P))
    S_total = int(caps.sum())
    T_total = S_total // P

    group_off = np.zeros(ngpc, dtype=np.int64)
    group_off[1:] = np.cumsum(caps)[:-1]

    starts = np.zeros(ngroups + 1, dtype=np.int64)
    starts[1:] = np.cumsum(counts.reshape(-1))
    within = np.arange(len(key_s)) - starts[key_s]
    slot = group_off[key_s % ngpc] + within
    ecore = key_s // ngpc

    idx_pad = np.zeros((NCORES, S_total), dtype=np.int16)
    rloc_pad = np.zeros((NCORES, S_total), dtype=np.float16)
    val_pad = np.zeros((NCORES, S_total), dtype=np.float16)
    idx_pad[ecore, slot] = qcol_s.astype(np.int16)
    rloc_pad[ecore, slot] = rloc_s.astype(np.float16)
    val_pad[ecore, slot] = vals_s.astype(np.float16)

    # gather table: x in fp16, 256B pitch; only cols 0-63 are ever read
    xtab = np.zeros((N, 128), dtype=np.float16)
    xtab[:, :C] = x.astype(np.float16)

    wcwl = np.concatenate([w_conv, w_lin], axis=1).astype(np.float16)

    in_maps = []
    for c in range(NCORES):
        idx_w = np.tile(
            np.ascontiguousarray(idx_pad[c].reshape(S_total // 16, 16).T),
            (8, 1))
        rl_w = np.ascontiguousarray(rloc_pad[c].reshape(T_total, P).T)
        vl_w = np.ascontiguousarray(val_pad[c].reshape(T_total, P).T)
        xT = np.zeros((C, RPAD), dtype=np.float16)
        xT[:, :R] = x[c * R:(c + 1) * R].T.astype(np.float16)
        in_maps.append({
            "xtab": xtab,
            "idx": np.ascontiguousarray(idx_w),
            "rl": rl_w,
            "vl": vl_w,
            "xt": xT,
            "w": np.ascontiguousarray(wcwl),
        })
    return in_maps, caps.reshape(NSB, NQ, SB)


# ---------------------------------------------------------------- device IR

def _build(caps):
    caps = np.asarray(caps)                     # [NSB, NQ, SB]
    nsb = int(os.environ.get("K_NSB", NSB))
    S_total = int(caps.sum())
    T_total = S_total // P
    tiles_sb = caps.sum(axis=(1, 2)) // P       # tiles per superblock
    T_max = int(tiles_sb.max())
    W_max = int((caps.sum(axis=(1, 2)) // 16).max())
    SCH = (T_max + 3) // 4                      # one-hot chunk tiles
    OGRP = 8  # output blocks staged per out DMA

    nc = bacc.Bacc("TRN2", target_bir_lowering=False, debug=False,
                   enable_asserts=False, num_devices=NCORES,
                   num_swdge_queues=4)
    xtab = nc.dram_tensor("xtab", [N, 128], mybir.dt.float16,
                          kind="ExternalInput").ap()
    idx_d = nc.dram_tensor("idx", [P, S_total // 16], mybir.dt.int16,
                           kind="ExternalInput").ap()
    rl_d = nc.dram_tensor("rl", [P, T_total], mybir.dt.float16,
                          kind="ExternalInput").ap()
    vl_d = nc.dram_tensor("vl", [P, T_total], mybir.dt.float16,
                          kind="ExternalInput").ap()
    xt_d = nc.dram_tensor("xt", [C, RPAD], mybir.dt.float16,
                          kind="ExternalInput").ap()
    w_d = nc.dram_tensor("w", [C, 2 * C], mybir.dt.float16,
                         kind="ExternalInput").ap()
    out_d = nc.dram_tensor("out", [P, NBLK, C], mybir.dt.float32,
                           kind="ExternalOutput").ap()

    with tile.TileContext(nc) as tc:
        with tc.tile_pool(name="const", bufs=1) as cpool, \
             tc.tile_pool(name="gb", bufs=2) as gpool, \
             tc.tile_pool(name="g2", bufs=2) as g2pool, \
             tc.tile_pool(name="meta", bufs=2) as mpool, \
             tc.tile_pool(name="oh", bufs=6) as ohpool, \
             tc.tile_pool(name="stg", bufs=2) as spool, \
             tc.tile_pool(name="ps1", bufs=2, space="PSUM") as ps1, \
             tc.tile_pool(name="ps2", bufs=2, space="PSUM") as ps2:

            # constants
            iota_i = cpool.tile([P, P], mybir.dt.int16)
            nc.gpsimd.iota(iota_i[:], pattern=[[1, P]], base=0,
                           channel_multiplier=0)
            iota_f = cpool.tile([P, P], mybir.dt.float16)
            nc.vector.tensor_copy(iota_f[:], iota_i[:])
            w_t = cpool.tile([C, 2 * C], mybir.dt.float16)
            nc.sync.dma_start(w_t[:], w_d)
            xt_t = cpool.tile([C, RPAD], mybir.dt.float16)
            nc.sync.dma_start(xt_t[:], xt_d)

            slot_off = 0   # entries consumed so far
            tile_off = 0   # entry-tiles consumed so far
            ob = None
            for s in range(nsb):
                k_sb = min(SB, NBLK - s * SB)          # blocks in this sb
                T_s = int(tiles_sb[s])
                W_s = int(caps[s].sum() // 16)

                idx_t = mpool.tile([P, W_max], mybir.dt.int16, tag="idx")
                nc.sync.dma_start(
                    idx_t[:, :W_s],
                    idx_d[:, slot_off // 16: slot_off // 16 + W_s])
                rl_t = mpool.tile([P, T_max], mybir.dt.float16, tag="rl")
                nc.sync.dma_start(rl_t[:, :T_s],
                                  rl_d[:, tile_off: tile_off + T_s])
                vl_t = mpool.tile([P, T_max], mybir.dt.float16, tag="vl")
                nc.sync.dma_start(vl_t[:, :T_s],
                                  vl_d[:, tile_off: tile_off + T_s])

                gbuf = gpool.tile([P, T_max, C], mybir.dt.float16, tag="g")
                r0 = 0
                for qq in range(NQ):
                    cq = int(caps[s, qq].sum())        # idxs this call
                    if cq == 0:
                        continue
                    _dma_gather_128(
                        nc.gpsimd,
                        gbuf[:, r0:r0 + cq // P, :],
                        xtab[qq * QROWS:(qq + 1) * QROWS, :C],
                        idx_t[:, r0 * 8: r0 * 8 + cq // 16],
                        cq, C, 128, queue_num=qq)
                    r0 += cq // P

                # batched one-hot: S[p, t, j] = (iota[j] == rloc[p, t])
                n_ch = (T_s + SCH - 1) // SCH
                schunks = []
                for ci in range(n_ch):
                    t0 = ci * SCH
                    tn = min(SCH, T_s - t0)
                    sbig = ohpool.tile([P, SCH, P], mybir.dt.float16,
                                       tag="oh")
                    nc.vector.tensor_tensor(
                        out=sbig[:, :tn, :],
                        in0=iota_f[:, None, :].broadcast_to((P, tn, P)),
                        in1=rl_t[:, t0:t0 + tn, None].broadcast_to((P, tn, P)),
                        op=mybir.AluOpType.is_equal)
                    schunks.append(sbig)

                # val-scale the gathered rows: G2 = G * val
                g2 = g2pool.tile([P, T_max, C], mybir.dt.float16, tag="g2")
                nc.vector.tensor_tensor(
                    out=g2[:, :T_s, :],
                    in0=gbuf[:, :T_s, :],
                    in1=vl_t[:, :T_s, None].broadcast_to((P, T_s, C)),
                    op=mybir.AluOpType.mult)

                # segment-sum all tiles into one PSUM bank [64, SB*128]
                psum_sT = ps1.tile([C, SB * P], mybir.dt.float32)
                tile_blocks = []
                for qq in range(NQ):
                    for bb in range(SB):
                        tile_blocks += [bb] * (int(caps[s, qq, bb]) // P)
                for t, bb in enumerate(tile_blocks):
                    nc.tensor.matmul(
                        psum_sT[:, bb * P:(bb + 1) * P],
                        g2[:, t, :],
                        schunks[t // SCH][:, t % SCH, :],
                        start=(t == 0),
                        stop=(t == len(tile_blocks) - 1),
                    )

                for bb in range(k_sb):
                    b = s * SB + bb
                    sT_sb = spool.tile([C, P], mybir.dt.float16, tag="sT")
                    nc.scalar.copy(sT_sb[:], psum_sT[:, bb * P:(bb + 1) * P])

                    out2 = ps2.tile([P, C], mybir.dt.float32)
                    nc.tensor.matmul(out2[:], sT_sb[:], w_t[:, 0:C],
                                     start=True, stop=False)
                    nc.tensor.matmul(out2[:], xt_t[:, b * P:(b + 1) * P],
                                     w_t[:, C:2 * C], start=False, stop=True)

                    g = b // OGRP
                    j = b % OGRP
                    gsz = min(OGRP, NBLK - g * OGRP)
                    if j == 0:
                        ob = spool.tile([P, OGRP, C], mybir.dt.float32,
                                        tag="ob")
                    nc.scalar.activation(ob[:, j, :], out2[:],
                                         mybir.ActivationFunctionType.Sigmoid)
                    if j == gsz - 1:
                        nc.sync.dma_start(
                            out_d[:, g * OGRP:g * OGRP + gsz, :],
                            ob[:, :gsz, :])

                slot_off += int(caps[s].sum())
                tile_off += T_s
    nc.compile()
    return nc


# ---------------------------------------------------------------- entry

_CACHE = {}


def _prepare(inputs):
    in_maps, caps = _preprocess(inputs)
    key = caps.tobytes()
    if key not in _CACHE:
        _CACHE[key] = _build(caps)
    return _CACHE[key], in_maps


def kernel(**inputs):
    nc, in_maps = _prepare(inputs)
    res = bass_utils.run_bass_kernel_spmd(nc, in_maps,
                                          core_ids=list(range(NCORES)))
    outs = []
    for c in range(NCORES):
        o = res.results[c]["out"]          # [P, NBLK, C]
        outs.append(o.transpose(1, 0, 2).reshape(RPAD, C)[:R])
    return np.concatenate(outs, axis=0).astype(np.float32)


# revision 10
# speedup vs baseline: 1.0043x; 1.0043x over previous
"""CANLayer (GNN message passing) Trainium2 kernel — 8 NeuronCores.

y = sigmoid(L_down @ (x Wc) + L_up @ (x Wc) + x Wl)

v2 design (self-contained: full inputs in, full output out):
  - segment_sum commutes with the dense right-multiplication by Wc: gather raw
    x rows per COO entry, scale by val, segment-sum per 128-row dest block on
    the PE, then apply Wc/Wl per block.
  - dest rows sharded across 8 cores (12500 each); entries bucketed by
    (superblock of 4 dest blocks, source quarter, block) on the host.
  - gather: 128B elements (64ch fp16) from a 256B-pitch table — half the HBM
    traffic of the 256B-elem minimum that bass' dma_gather wrapper enforces
    (the Q7 kernel itself only needs the 256B stride alignment).
  - gathers for quarter q issue on SWDGE queue q: queues 1-3 run their
    descriptor generation on GPSIMD core-pairs (2,3),(4,5),(6,7) in the
    background, so 4 quarters' descgen runs concurrently (~8ns/desc/pair).
  - one-hot S[e, pos] = (iota == rloc_e) is built in BATCHED tensor_tensor
    is_equal ops over a quarter-superblock span using stride-0 broadcast APs
    for rloc — ~2 DVE cycles/elem instead of a per-tile tensor_scalar.
  - val is folded into the gathered rows (G2 = G * val, batched broadcast
    mult), so the one-hot is a pure 0/1 matrix.
  - PE per entry-tile: psum_sT[64, 4*128] += G2_t[:, :64].T @ S_t (one
    accumulation group per superblock bank).
"""
import os

import numpy as np

import concourse.mybir as mybir
import concourse.tile as tile
from concourse import bacc
from concourse import bass_utils
from concourse import ap_utils
from concourse._compat import round_up_to_multiple

N = 100000
C = 64
NCORES = 8
P = 128
R = N // NCORES            # 12500 rows per core
NBLK = (R + P - 1) // P    # 98 blocks per core
RPAD = NBLK * P            # 12544
NQ = 4
QROWS = N // NQ            # 25000
SB = 4                     # dest blocks per superblock (one PSUM bank)
NSB = (NBLK + SB - 1) // SB


def _exact_div(a, b):
    assert a % b == 0
    return a // b


def _dma_gather_128(eng, out_ap, in_ap, idxs_ap, num_idxs, elem_size,
                    elem_step, queue_num=0):
    """dma_gather with sub-256B elements (stride still 256B-aligned).

    Mirrors concourse.bass dma_gather minus its `elem_size_bytes % 256`
    assert, which is a transpose-path restriction the non-transpose Q7
    kernel does not need (verified bit-exact vs the 256B path on HW).
    """
    assert idxs_ap.dtype == mybir.dt.int16
    assert in_ap.dtype == out_ap.dtype
    assert ap_utils.ap_is_contiguous(out_ap.ap[1:])
    assert ap_utils.ap_is_contiguous(idxs_ap.ap[1:])
    assert in_ap.ap[-1][1] == elem_size == out_ap.ap[-1][1]
    assert out_ap.ap[0][1] * out_ap.ap[1][1] == round_up_to_multiple(num_idxs, 128)
    assert in_ap.ap[0][0] == elem_step
    stride_bytes = elem_step * mybir.dt.size(in_ap.dtype)
    stride_bytes_256 = _exact_div(stride_bytes, 256)
    _in_ap = eng.lower_ap_dma(in_ap, for_custom_bir_dma=True)
    return eng.add_instruction(
        mybir.InstDMAGatherAnt(
            name=eng.bass.get_next_instruction_name(),
            ins=[*_in_ap, eng.lower_ap(idxs_ap),
                 eng.lower_val_access(eng.to_reg(num_idxs))],
            outs=[eng.lower_ap(out_ap)],
            transpose=False, num_idxs=num_idxs, elem_size=elem_size,
            stride_bytes_256=stride_bytes_256, gen_mode=0,
            single_packet=False, queue_num=queue_num,
            sbuf_tokens_per_rank=0, sbuf_free_dim_per_rank=0,
            sbuf_free_dim_pad_per_rank=0, sbuf_byte_offset=0))


# ---------------------------------------------------------------- host prep

def _preprocess(inputs):
    x = np.ascontiguousarray(np.asarray(inputs["x"], dtype=np.float32))
    w_conv = np.asarray(inputs["w_conv"], dtype=np.float32)
    w_lin = np.asarray(inputs["w_lin"], dtype=np.float32)

    rows = np.concatenate([np.asarray(inputs["down_rows"]),
                           np.asarray(inputs["up_rows"])]).astype(np.int64)
    cols = np.concatenate([np.asarray(inputs["down_cols"]),
                           np.asarray(inputs["up_cols"])]).astype(np.int64)
    vals = np.concatenate([np.asarray(inputs["down_vals"]),
                           np.asarray(inputs["up_vals"])]).astype(np.float32)

    core = rows // R
    rl = rows % R
    blk = rl // P
    rloc = rl - blk * P
    q = cols // QROWS
    qcol = cols - q * QROWS

    # group order: (core, superblock, quarter, block-in-superblock)
    sb = blk // SB
    bin_ = blk - sb * SB
    gkey = (sb * NQ + q) * SB + bin_            # within-core group id
    ngpc = NSB * NQ * SB                        # groups per core (incl ghosts)
    key = core * ngpc + gkey
    order = np.argsort(key, kind="stable")
    key_s = key[order]
    qcol_s = qcol[order]
    rloc_s = rloc[order]
    vals_s = vals[order]

    ngroups = NCORES * ngpc
    counts = np.bincount(key_s, minlength=ngroups).reshape(NCORES, ngpc)
    caps = counts.max(axis=0)                   # [ngpc]
    # ghost groups (blocks beyond NBLK in the last superblock) stay size 0
    g_ids = np.arange(ngpc)
    g_blk = (g_ids // (NQ * SB)) * SB + (g_ids % SB)
    ghost = g_blk >= NBLK
    caps = np.where(ghost, 0, np.maximum(((caps + P - 1) // P) * P, P))
    S_total = int(caps.sum())
    T_total = S_total // P

    group_off = np.zeros(ngpc, dtype=np.int64)
    group_off[1:] = np.cumsum(caps)[:-1]

    starts = np.zeros(ngroups + 1, dtype=np.int64)
    starts[1:] = np.cumsum(counts.reshape(-1))
    within = np.arange(len(key_s)) - starts[key_s]
    slot = group_off[key_s % ngpc] + within
    ecore = key_s // ngpc

    idx_pad = np.zeros((NCORES, S_total), dtype=np.int16)
    rloc_pad = np.zeros((NCORES, S_total), dtype=np.float16)
    val_pad = np.zeros((NCORES, S_total), dtype=np.float16)
    idx_pad[ecore, slot] = qcol_s.astype(np.int16)
    rloc_pad[ecore, slot] = rloc_s.astype(np.float16)
    val_pad[ecore, slot] = vals_s.astype(np.float16)

    # gather table: x in fp16, 256B pitch; only cols 0-63 are ever read
    xtab = np.zeros((N, 128), dtype=np.float16)
    xtab[:, :C] = x.astype(np.float16)

    wcwl = np.concatenate([w_conv, w_lin], axis=1).astype(np.float16)

    in_maps = []
    for c in range(NCORES):
        idx_w = np.tile(
            np.ascontiguousarray(idx_pad[c].reshape(S_total // 16, 16).T),
            (8, 1))
        rl_w = np.ascontiguousarray(rloc_pad[c].reshape(T_total, P).T)
        vl_w = np.ascontiguousarray(val_pad[c].reshape(T_total, P).T)
        xT = np.zeros((C, RPAD), dtype=np.float16)
        xT[:, :R] = x[c * R:(c + 1) * R].T.astype(np.float16)
        in_maps.append({
            "xtab": xtab,
            "idx": np.ascontiguousarray(idx_w),
            "rl": rl_w,
            "vl": vl_w,
            "xt": xT,
            "w": np.ascontiguousarray(wcwl),
        })
    return in_maps, caps.reshape(NSB, NQ, SB)


# ---------------------------------------------------------------- device IR

def _build(caps):
    caps = np.asarray(caps)                     # [NSB, NQ, SB]
    nsb = int(os.environ.get("K_NSB", NSB))
    S_total = int(caps.sum())
    T_total = S_total // P
    tiles_sb = caps.sum(axis=(1, 2)) // P       # tiles per superblock
    T_max = int(tiles_sb.max())
    W_max = int((caps.sum(axis=(1, 2)) // 16).max())
    SCH = (((T_max + 3) // 4) + 1) & ~1         # one-hot chunk tiles (even)
    OGRP = 8  # output blocks staged per out DMA

    nc = bacc.Bacc("TRN2", target_bir_lowering=False, debug=False,
                   enable_asserts=False, num_devices=NCORES,
                   num_swdge_queues=4)
    xtab = nc.dram_tensor("xtab", [N, 128], mybir.dt.float16,
                          kind="ExternalInput").ap()
    idx_d = nc.dram_tensor("idx", [P, S_total // 16], mybir.dt.int16,
                           kind="ExternalInput").ap()
    rl_d = nc.dram_tensor("rl", [P, T_total], mybir.dt.float16,
                          kind="ExternalInput").ap()
    vl_d = nc.dram_tensor("vl", [P, T_total], mybir.dt.float16,
                          kind="ExternalInput").ap()
    xt_d = nc.dram_tensor("xt", [C, RPAD], mybir.dt.float16,
                          kind="ExternalInput").ap()
    w_d = nc.dram_tensor("w", [C, 2 * C], mybir.dt.float16,
                         kind="ExternalInput").ap()
    out_d = nc.dram_tensor("out", [P, NBLK, C], mybir.dt.float32,
                           kind="ExternalOutput").ap()

    with tile.TileContext(nc) as tc:
        with tc.tile_pool(name="const", bufs=1) as cpool, \
             tc.tile_pool(name="gb", bufs=2) as gpool, \
             tc.tile_pool(name="g2", bufs=2) as g2pool, \
             tc.tile_pool(name="meta", bufs=2) as mpool, \
             tc.tile_pool(name="oh", bufs=6) as ohpool, \
             tc.tile_pool(name="stg", bufs=2) as spool, \
             tc.tile_pool(name="ps1", bufs=2, space="PSUM") as ps1, \
             tc.tile_pool(name="ps2", bufs=2, space="PSUM") as ps2:

            # constants
            iota_i = cpool.tile([P, P], mybir.dt.int16)
            nc.gpsimd.iota(iota_i[:], pattern=[[1, P]], base=0,
                           channel_multiplier=0)
            iota_f = cpool.tile([P, P], mybir.dt.float16)
            nc.vector.tensor_copy(iota_f[:], iota_i[:])
            # iota replicated along a trailing tile axis: ich[p, j, t] = j.
            # Dense (step-1 innermost) so the batched is_eq qualifies for
            # the DVE 2x packed mode (stride-0 innermost forces 1x).
            ich = cpool.tile([P, P, SCH], mybir.dt.float16)
            nc.vector.tensor_copy(
                ich[:], iota_f[:, :, None].broadcast_to((P, P, SCH)))
            w_t = cpool.tile([C, 2 * C], mybir.dt.float16)
            nc.sync.dma_start(w_t[:], w_d)
            xt_t = cpool.tile([C, RPAD], mybir.dt.float16)
            nc.sync.dma_start(xt_t[:], xt_d)

            slot_off = 0   # entries consumed so far
            tile_off = 0   # entry-tiles consumed so far
            ob = None
            for s in range(nsb):
                k_sb = min(SB, NBLK - s * SB)          # blocks in this sb
                T_s = int(tiles_sb[s])
                W_s = int(caps[s].sum() // 16)

                idx_t = mpool.tile([P, W_max], mybir.dt.int16, tag="idx")
                nc.sync.dma_start(
                    idx_t[:, :W_s],
                    idx_d[:, slot_off // 16: slot_off // 16 + W_s])
                rl_t = mpool.tile([P, T_max + 2], mybir.dt.float16, tag="rl")
                nc.sync.dma_start(rl_t[:, :T_s],
                                  rl_d[:, tile_off: tile_off + T_s])
                vl_t = mpool.tile([P, T_max], mybir.dt.float16, tag="vl")
                nc.sync.dma_start(vl_t[:, :T_s],
                                  vl_d[:, tile_off: tile_off + T_s])

                gbuf = gpool.tile([P, T_max, C], mybir.dt.float16, tag="g")
                r0 = 0
                for qq in range(NQ):
                    cq = int(caps[s, qq].sum())        # idxs this call
                    if cq == 0:
                        continue
                    _dma_gather_128(
                        nc.gpsimd,
                        gbuf[:, r0:r0 + cq // P, :],
                        xtab[qq * QROWS:(qq + 1) * QROWS, :C],
                        idx_t[:, r0 * 8: r0 * 8 + cq // 16],
                        cq, C, 128, queue_num=qq)
                    r0 += cq // P

                # batched one-hot: S[p, j, t] = (iota[j] == rloc[p, t]).
                # [j, t] layout keeps both inputs' innermost dims step-1
                # (2x DVE mode); the MM rhs reads column-strided slices.
                n_ch = (T_s + SCH - 1) // SCH
                schunks = []
                for ci in range(n_ch):
                    t0 = ci * SCH
                    tn = (min(SCH, T_s - t0) + 1) & ~1
                    sbig = ohpool.tile([P, P, SCH], mybir.dt.float16,
                                       tag="oh")
                    nc.vector.tensor_tensor(
                        out=sbig[:, :, :tn],
                        in0=ich[:, :, :tn],
                        in1=rl_t[:, None, t0:t0 + tn].broadcast_to((P, P, tn)),
                        op=mybir.AluOpType.is_equal)
                    schunks.append(sbig)

                # val-scale the gathered rows: G2 = G * val
                g2 = g2pool.tile([P, T_max, C], mybir.dt.float16, tag="g2")
                nc.vector.tensor_tensor(
                    out=g2[:, :T_s, :],
                    in0=gbuf[:, :T_s, :],
                    in1=vl_t[:, :T_s, None].broadcast_to((P, T_s, C)),
                    op=mybir.AluOpType.mult)

                # segment-sum all tiles into one PSUM bank [64, SB*128]
                psum_sT = ps1.tile([C, SB * P], mybir.dt.float32)
                tile_blocks = []
                for qq in range(NQ):
                    for bb in range(SB):
                        tile_blocks += [bb] * (int(caps[s, qq, bb]) // P)
                for t, bb in enumerate(tile_blocks):
                    nc.tensor.matmul(
                        psum_sT[:, bb * P:(bb + 1) * P],
                        g2[:, t, :],
                        schunks[t // SCH][:, :, t % SCH],
                        start=(t == 0),
                        stop=(t == len(tile_blocks) - 1),
                    )

                for bb in range(k_sb):
                    b = s * SB + bb
                    sT_sb = spool.tile([C, P], mybir.dt.float16, tag="sT")
                    nc.scalar.copy(sT_sb[:], psum_sT[:, bb * P:(bb + 1) * P])

                    out2 = ps2.tile([P, C], mybir.dt.float32)
                    nc.tensor.matmul(out2[:], sT_sb[:], w_t[:, 0:C],
                                     start=True, stop=False)
                    nc.tensor.matmul(out2[:], xt_t[:, b * P:(b + 1) * P],
                                     w_t[:, C:2 * C], start=False, stop=True)

                    g = b // OGRP
                    j = b % OGRP
                    gsz = min(OGRP, NBLK - g * OGRP)
                    if j == 0:
                        ob = spool.tile([P, OGRP, C], mybir.dt.float32,
                                        tag="ob")
                    nc.scalar.activation(ob[:, j, :], out2[:],
                                         mybir.ActivationFunctionType.Sigmoid)
                    if j == gsz - 1:
                        nc.sync.dma_start(
                            out_d[:, g * OGRP:g * OGRP + gsz, :],
                            ob[:, :gsz, :])

                slot_off += int(caps[s].sum())
                tile_off += T_s
    nc.compile()
    return nc


# ---------------------------------------------------------------- entry

_CACHE = {}


def _prepare(inputs):
    in_maps, caps = _preprocess(inputs)
    key = caps.tobytes()
    if key not in _CACHE:
        _CACHE[key] = _build(caps)
    return _CACHE[key], in_maps


def kernel(**inputs):
    nc, in_maps = _prepare(inputs)
    res = bass_utils.run_bass_kernel_spmd(nc, in_maps,
                                          core_ids=list(range(NCORES)))
    outs = []
    for c in range(NCORES):
        o = res.results[c]["out"]          # [P, NBLK, C]
        outs.append(o.transpose(1, 0, 2).reshape(RPAD, C)[:R])
    return np.concatenate(outs, axis=0).astype(np.float32)
